# revision 1
# baseline (speedup 1.0000x reference)
"""Trainium2 Bass kernel for a 2-layer char-LSTM (B=64, T=512, H=1024, V=256).

Strategy:
- 8-way tensor-parallel over the 4H gate dimension: core c owns gate columns
  [i_c | f_c | o_c | g_c] (128 each, gate-reordered so sigmoid covers one
  contiguous 384-col block), and hidden-state chunk c (128 dims) per layer.
- The embedding + layer-0 input projection fold into a row gather of
  Wie = embed @ Wi0_c + b0_c, implemented as one-hot matmuls accumulated
  straight into the gate PSUM (exact math, no explicit embedding tensor).
- Per step, each core computes its 512 gate columns, updates its 128-dim
  h/c chunk, transposes h to (hid, batch) layout and the chunks are
  all-gathered across cores for the next step's matmuls.
- The vocab projection is computed redundantly on every core (it is free
  under the comm-bound wave period) so the SPMD program stays uniform.
- Matmuls run in float32r (full-rate fp32 path, ~1.5e-4 rel err measured).
"""
import sys

sys.path.insert(0, "/opt/trn_rl_repo")

import numpy as np
from concourse import bacc, tile, mybir
from concourse.bass import ds as bass_ds, _add_dep_helper
from concourse.bass_utils import run_bass_kernel_spmd

B, T, H, V, NC = 64, 512, 1024, 256, 8
KT = H // 128            # 8 contraction tiles of 128
HC = H // NC             # 128 hidden dims per core
GC = 4 * H // NC         # 512 gate cols per core
CH = 16                  # steps per one-hot build chunk

DT = mybir.dt.float32
DTR = mybir.dt.float32r
DTI = mybir.dt.int32
AF = mybir.ActivationFunctionType
ALU = mybir.AluOpType


def r(ap):
    return ap.bitcast(DTR)


def build_nc(t_steps=T, reps=1, comm="ag"):
    nc = bacc.Bacc(None, target_bir_lowering=False, num_devices=NC)

    p_embed = nc.declare_dram_parameter("embed", [V, H], DTR, isOutput=False)
    p_wi0 = nc.declare_dram_parameter("wi0", [128, KT * GC], DTR, isOutput=False)
    p_wh0 = nc.declare_dram_parameter("wh0", [128, KT * GC], DTR, isOutput=False)
    p_wi1 = nc.declare_dram_parameter("wi1", [128, KT * GC], DTR, isOutput=False)
    p_wh1 = nc.declare_dram_parameter("wh1", [128, KT * GC], DTR, isOutput=False)
    p_b0 = nc.declare_dram_parameter("b0", [1, GC], DTR, isOutput=False)
    p_b1 = nc.declare_dram_parameter("b1", [1, GC], DTR, isOutput=False)
    p_wproj = nc.declare_dram_parameter("wproj", [128, KT * V], DTR, isOutput=False)
    p_oh = nc.declare_dram_parameter(
        "onehot", [2, 128, t_steps * B], DTR, isOutput=False
    )
    p_out = nc.declare_dram_parameter("out", [B, t_steps, V], DT, isOutput=True)

    c_ident = nc.inline_tensor(np.eye(128, dtype=np.float32), name="ident")
    c_ones = nc.inline_tensor(np.ones((1, B), dtype=np.float32), name="ones")
    c_zero = nc.inline_tensor(
        np.zeros((128, 2 * KT * B), dtype=np.float32), name="zeros"
    )

    from contextlib import ExitStack

    with tile.TileContext(nc) as tc, ExitStack() as stack:
        wp = stack.enter_context(tc.tile_pool(name="weights", bufs=1))
        wh0_sb = wp.tile([128, KT * GC], DTR, tag="wh0")
        wi1_sb = wp.tile([128, KT * GC], DTR, tag="wi1")
        wh1_sb = wp.tile([128, KT * GC], DTR, tag="wh1")
        wproj_sb = wp.tile([128, KT * V], DTR, tag="wproj")
        wie_sb = wp.tile([128, 2 * GC], DTR, tag="wie")
        b1_sb = wp.tile([1, GC], DTR, tag="b1")
        b0_sb = wp.tile([1, GC], DTR, tag="b0")
        ident_sb = wp.tile([128, 128], DTR, tag="ident")
        ones_sb = wp.tile([1, B], DTR, tag="ones")

        nc.sync.dma_start(wh0_sb[:], p_wh0[:])
        nc.sync.dma_start(wi1_sb[:], p_wi1[:])
        nc.sync.dma_start(wh1_sb[:], p_wh1[:])
        nc.sync.dma_start(wproj_sb[:], p_wproj[:])
        nc.sync.dma_start(b1_sb[:], p_b1[:])
        nc.sync.dma_start(b0_sb[:], p_b0[:])
        nc.gpsimd.dma_start(ident_sb[:], c_ident[:])
        nc.gpsimd.dma_start(ones_sb[:], c_ones[:])

        # ---- prologue: Wie = embed @ Wi0_c + b0_c, via on-device transpose
        with (
            tc.tile_pool(name="prolog", bufs=1) as pp,
            tc.tile_pool(name="prolog_ps", bufs=2, space="PSUM") as pps,
        ):
            wi0_sb = pp.tile([128, KT * GC], DTR, tag="wi0")
            em_sb = pp.tile([128, 2 * H], DTR, tag="em")  # vocab halves
            emt_sb = pp.tile([128, KT * V], DTR, tag="emt")
            nc.sync.dma_start(wi0_sb[:], p_wi0[:])
            nc.sync.dma_start(em_sb[:, 0:H], p_embed[0:128, :])
            nc.sync.dma_start(em_sb[:, H : 2 * H], p_embed[128:V, :])
            for k in range(KT):
                for vh in range(2):
                    pt = pps.tile([128, 128], DTR, tag="ptr")
                    nc.tensor.transpose(
                        r(pt[:]),
                        r(em_sb[:, vh * H + k * 128 : vh * H + (k + 1) * 128]),
                        r(ident_sb[:]),
                    )
                    nc.vector.tensor_copy(
                        emt_sb[:, k * V + vh * 128 : k * V + (vh + 1) * 128], pt[:]
                    )
            for m in range(2):
                ps = pps.tile([128, GC], DT, tag="pwie")
                for k in range(KT):
                    nc.tensor.matmul(
                        ps[:],
                        r(emt_sb[:, k * V + m * 128 : k * V + (m + 1) * 128]),
                        r(wi0_sb[:, k * GC : (k + 1) * GC]),
                        start=(k == 0),
                        stop=(k == KT - 1),
                    )
                nc.vector.tensor_copy(wie_sb[:, m * GC : (m + 1) * GC], ps[:])

        # ---- main loop pools
        hT = stack.enter_context(tc.tile_pool(name="hT", bufs=3))
        cst = stack.enter_context(tc.tile_pool(name="cstate", bufs=3))
        oh = stack.enter_context(tc.tile_pool(name="onehot", bufs=2))
        gp = stack.enter_context(tc.tile_pool(name="gates", bufs=3))
        tp = stack.enter_context(tc.tile_pool(name="tmp", bufs=4))
        dr = stack.enter_context(tc.tile_pool(name="dram", bufs=3, space="DRAM"))
        zp = stack.enter_context(tc.tile_pool(name="zpsum", bufs=2, space="PSUM"))
        tps = stack.enter_context(tc.tile_pool(name="tpsum", bufs=2, space="PSUM"))
        pps2 = stack.enter_context(tc.tile_pool(name="ppsum", bufs=2, space="PSUM"))

        def gather(own_sb):
            """AllGather (128, B) chunks -> (128, KT*B) k-tile layout."""
            agin = dr.tile([128, B], DTR, tag="agin")
            agout = dr.tile(
                [KT * 128, B], DTR, tag="agout",
                addr_space="Shared" if comm == "ag_shared" else "Local",
            )
            nc.sync.dma_start(agin[:], own_sb[:])
            if comm != "local":
                nc.gpsimd.collective_compute(
                    "AllGather",
                    ALU.bypass,
                    replica_groups=[list(range(NC))],
                    ins=[agin[:].opt()],
                    outs=[agout[:].opt()],
                )
            else:
                nc.sync.dma_start(agout[0:128, :], agin[:])
            nxt = hT.tile([128, KT * B], DTR, tag="hT")
            src = agout[:].rearrange("(k p) j -> p k j", k=KT, p=128, j=B)
            dst = nxt[:].rearrange("p (k j) -> p k j", k=KT, j=B)
            nc.sync.dma_start(dst, src)
            return nxt

        def cell(z, c_prev, tagpfx, dst=None):
            """Gate math in (batch, gate) layout. Returns (h_sb, c_new)."""
            ga = gp.tile([64, GC], DT, tag=tagpfx + "ga")
            nc.scalar.activation(ga[:], z[:], AF.Tanh)
            sg = gp.tile([64, 384], DT, tag=tagpfx + "sg")
            nc.vector.tensor_scalar(
                sg[:], ga[:, 0:384], 0.5, 0.5, ALU.mult, ALU.add
            )
            ig = tp.tile([64, HC], DT, tag=tagpfx + "ig")
            nc.vector.tensor_tensor(ig[:], sg[:, 0:128], ga[:, 384:512], ALU.mult)
            cf = tp.tile([64, HC], DT, tag=tagpfx + "cf")
            nc.vector.tensor_tensor(cf[:], c_prev[:], sg[:, 128:256], ALU.mult)
            c_new = cst.tile([64, HC], DT, tag=tagpfx + "c")
            nc.vector.tensor_tensor(c_new[:], ig[:], cf[:], ALU.add)
            th = tp.tile([64, HC], DT, tag=tagpfx + "th")
            nc.scalar.activation(th[:], c_new[:], AF.Tanh)
            h = tp.tile([64, HC], DTR, tag=tagpfx + "h")
            nc.vector.tensor_tensor(h[:], sg[:, 256:384], th[:], ALU.mult)
            ptp = tps.tile([128, B], DTR, tag="ptp")
            nc.tensor.transpose(r(ptp[:]), r(h[:]), r(ident_sb[0:64, 0:64]))
            if dst is None:
                hTo = tp.tile([128, B], DTR, tag=tagpfx + "hT")
                nc.vector.tensor_copy(hTo[:], ptp[:])
                return hTo, c_new
            nc.vector.tensor_copy(dst, ptp[:])
            return None, c_new

        ob_pool = stack.enter_context(tc.tile_pool(name="obuf", bufs=4))
        harr = nc.alloc_semaphore("harr")
        lsnd = nc.alloc_semaphore("lsnd")
        gath_count = [0]
        deferred_waits = []
        fake_incs = []
        pid_cache = []

        def rdma_gather(ob):
            if not pid_cache:
                pid_cache.append(nc.gpsimd.partition_id())
            nxt = hT.tile([128, 2 * KT * B], DTR, tag="hBT")
            dst3 = nxt[:].rearrange("p (s j) -> p s j", s=NC, j=2 * B)
            out_ap = dst3[:, bass_ds(pid_cache[0], 1), :]
            nc.gpsimd.remote_dma_broadcast(
                out_ap,
                ob[:],
                harr,
                lsnd,
                rdests=[(0, k) for k in range(NC)],
            )
            trig = nc.gpsimd.trigger_dma(1)
            fk = nc.gpsimd.sem_inc(harr, 16)
            _add_dep_helper(fk.ins, trig.ins, True, "model remote arrival after trigger")
            fake_incs.append(fk)
            gath_count[0] += 1
            return nxt

        def fused_gather(o1, o2, w):
            agin = dr.tile([2 * 128, B], DTR, tag="agin")
            agout = dr.tile([2 * KT * 128, B], DTR, tag="agout")
            if o1 is not None:
                nc.sync.dma_start(agin[0:128, :], o1[:])
            else:
                nc.sync.dma_start(agin[0:128, :], c_zero[:, 0:B].bitcast(DTR))
            if o2 is not None:
                nc.sync.dma_start(agin[128:256, :], o2[:])
            else:
                nc.sync.dma_start(agin[128:256, :], c_zero[:, 0:B].bitcast(DTR))
            if comm != "wave_local":
                nc.gpsimd.collective_compute(
                    "AllGather",
                    ALU.bypass,
                    replica_groups=[list(range(NC))],
                    ins=[agin[:].opt()],
                    outs=[agout[:].opt()],
                )
            else:
                nc.sync.dma_start(agout[0:256, :], agin[:])
            nxt = hT.tile([128, 2 * KT * B], DTR, tag="hBT")
            src = agout[:].rearrange("(f p) j -> p f j", f=2 * KT, p=128)
            dst = nxt[:].rearrange("p (f j) -> p f j", f=2 * KT, j=B)
            nc.sync.dma_start(dst, src)
            return nxt

        def do_proj(hBT, t, thr=None, pin=None):
            pj = pps2.tile([64, V], DT, tag="pj")
            if thr is not None:
                src = pin if pin is not None else ones_sb
                dmy = nc.tensor.matmul(
                    pj[0:1, 0:2],
                    src[0:1, 0:1].bitcast(DT),
                    src[0:1, 0:2].bitcast(DT),
                    start=True, stop=True, skip_group_check=True,
                )
                dmy._wait_ge(harr, thr)
            for k in range(KT):
                nc.tensor.matmul(
                    pj[:],
                    hBT[:, (2 * k + 1) * B : (2 * k + 2) * B],
                    wproj_sb[:, k * V : (k + 1) * V],
                    start=(k == 0),
                    stop=(k == KT - 1),
                )
            lo = tp.tile([64, V], DT, tag="lo")
            nc.vector.tensor_copy(lo[:], pj[:])
            nc.sync.dma_start(p_out[:, t, :], lo[:])

        for _ in range(reps if comm == "rdma" else 0):
            hBT = hT.tile([128, 2 * KT * B], DTR, tag="hBT")
            nc.gpsimd.dma_start(hBT[:], c_zero[:])
            c1 = cst.tile([64, HC], DT, tag="1c")
            c2 = cst.tile([64, HC], DT, tag="2c")
            nc.vector.memset(c1[:], 0.0)
            nc.vector.memset(c2[:], 0.0)

            ohlo = ohhi = None
            prev_ob = None
            for w in range(t_steps + 1):
                wait_thr = 16 * gath_count[0] if w > 0 else None
                wait_thr2 = wait_thr
                ob = ob_pool.tile([128, 2 * B], DTR, tag="ob")
                if w == 0 or w == t_steps:
                    nc.gpsimd.dma_start(ob[:], c_zero[:, 0 : 2 * B].bitcast(DTR))
                if w < t_steps:
                    j = w % CH
                    if j == 0:
                        nch = min(CH, t_steps - w)
                        ohlo = oh.tile([128, CH * B], DTR, tag="ohlo")
                        ohhi = oh.tile([128, CH * B], DTR, tag="ohhi")
                        nc.sync.dma_start(
                            ohlo[:, 0 : nch * B], p_oh[0, :, w * B : (w + nch) * B]
                        )
                        nc.sync.dma_start(
                            ohhi[:, 0 : nch * B], p_oh[1, :, w * B : (w + nch) * B]
                        )
                    z1 = zp.tile([64, GC], DT, tag="z1")
                    if wait_thr is not None:
                        dmy = nc.tensor.matmul(
                            z1[0:1, 0:2],
                            prev_ob[0:1, 0:1].bitcast(DT),
                            prev_ob[0:1, 0:2].bitcast(DT),
                            start=True, stop=True, skip_group_check=True,
                        )
                        dmy._wait_ge(harr, wait_thr)
                    nc.tensor.matmul(
                        z1[:], ones_sb[:], b0_sb[:], start=True, stop=False
                    )
                    nc.tensor.matmul(
                        z1[:], ohlo[:, j * B : (j + 1) * B], wie_sb[:, 0:GC],
                        start=False, stop=False,
                    )
                    nc.tensor.matmul(
                        z1[:], ohhi[:, j * B : (j + 1) * B], wie_sb[:, GC : 2 * GC],
                        start=False, stop=False,
                    )
                    for k in range(KT):
                        nc.tensor.matmul(
                            z1[:],
                            hBT[:, 2 * k * B : (2 * k + 1) * B],
                            wh0_sb[:, k * GC : (k + 1) * GC],
                            start=False,
                            stop=(k == KT - 1),
                        )
                    _, c1 = cell(z1, c1, "1", dst=ob[:, 0:B])
                if w >= 1:
                    z2 = zp.tile([64, GC], DT, tag="z2")
                    if wait_thr2 is not None:
                        dmy = nc.tensor.matmul(
                            z2[0:1, 0:2],
                            prev_ob[0:1, 0:1].bitcast(DT),
                            prev_ob[0:1, 0:2].bitcast(DT),
                            start=True, stop=True, skip_group_check=True,
                        )
                        dmy._wait_ge(harr, wait_thr2)
                    nc.tensor.matmul(
                        z2[:], ones_sb[:], b1_sb[:], start=True, stop=False
                    )
                    for k in range(KT):
                        nc.tensor.matmul(
                            z2[:],
                            hBT[:, 2 * k * B : (2 * k + 1) * B],
                            wi1_sb[:, k * GC : (k + 1) * GC],
                            start=False,
                            stop=False,
                        )
                    for k in range(KT):
                        nc.tensor.matmul(
                            z2[:],
                            hBT[:, (2 * k + 1) * B : (2 * k + 2) * B],
                            wh1_sb[:, k * GC : (k + 1) * GC],
                            start=False,
                            stop=(k == KT - 1),
                        )
                    _, c2 = cell(z2, c2, "2", dst=ob[:, B : 2 * B])
                if w >= 2:
                    do_proj(hBT, w - 2, thr=16 * gath_count[0], pin=prev_ob)
                hBT = rdma_gather(ob)
                prev_ob = ob
            do_proj(hBT, t_steps - 1, thr=16 * gath_count[0], pin=prev_ob)

        for _ in range(reps if comm in ("wave", "wave_local") else 0):
            hBT = hT.tile([128, 2 * KT * B], DTR, tag="hBT")
            nc.gpsimd.dma_start(hBT[:], c_zero[:])
            c1 = cst.tile([64, HC], DT, tag="1c")
            c2 = cst.tile([64, HC], DT, tag="2c")
            nc.vector.memset(c1[:], 0.0)
            nc.vector.memset(c2[:], 0.0)

            ohlo = ohhi = None
            for w in range(t_steps + 1):
                o1 = o2 = None
                if w < t_steps:
                    j = w % CH
                    if j == 0:
                        nch = min(CH, t_steps - w)
                        ohlo = oh.tile([128, CH * B], DTR, tag="ohlo")
                        ohhi = oh.tile([128, CH * B], DTR, tag="ohhi")
                        nc.sync.dma_start(
                            ohlo[:, 0 : nch * B], p_oh[0, :, w * B : (w + nch) * B]
                        )
                        nc.sync.dma_start(
                            ohhi[:, 0 : nch * B], p_oh[1, :, w * B : (w + nch) * B]
                        )
                    z1 = zp.tile([64, GC], DT, tag="z1")
                    nc.tensor.matmul(
                        z1[:], ones_sb[:], b0_sb[:], start=True, stop=False
                    )
                    nc.tensor.matmul(
                        z1[:], ohlo[:, j * B : (j + 1) * B], wie_sb[:, 0:GC],
                        start=False, stop=False,
                    )
                    nc.tensor.matmul(
                        z1[:], ohhi[:, j * B : (j + 1) * B], wie_sb[:, GC : 2 * GC],
                        start=False, stop=False,
                    )
                    for k in range(KT):
                        nc.tensor.matmul(
                            z1[:],
                            hBT[:, 2 * k * B : (2 * k + 1) * B],
                            wh0_sb[:, k * GC : (k + 1) * GC],
                            start=False,
                            stop=(k == KT - 1),
                        )
                    o1, c1 = cell(z1, c1, "1")
                if w >= 1:
                    z2 = zp.tile([64, GC], DT, tag="z2")
                    nc.tensor.matmul(
                        z2[:], ones_sb[:], b1_sb[:], start=True, stop=False
                    )
                    for k in range(KT):
                        nc.tensor.matmul(
                            z2[:],
                            hBT[:, 2 * k * B : (2 * k + 1) * B],
                            wi1_sb[:, k * GC : (k + 1) * GC],
                            start=False,
                            stop=False,
                        )
                    for k in range(KT):
                        nc.tensor.matmul(
                            z2[:],
                            hBT[:, (2 * k + 1) * B : (2 * k + 2) * B],
                            wh1_sb[:, k * GC : (k + 1) * GC],
                            start=False,
                            stop=(k == KT - 1),
                        )
                    o2, c2 = cell(z2, c2, "2")
                if w >= 2:
                    do_proj(hBT, w - 2)
                hBT = fused_gather(o1, o2, w)
            do_proj(hBT, t_steps - 1)

        for _ in range(reps if comm not in ("wave", "wave_local") else 0):
            h1T = hT.tile([128, KT * B], DTR, tag="hT")
            h2T = hT.tile([128, KT * B], DTR, tag="hT")
            c1 = cst.tile([64, HC], DT, tag="1c")
            c2 = cst.tile([64, HC], DT, tag="2c")
            nc.gpsimd.dma_start(h1T[:], c_zero[:, 0 : KT * B])
            nc.gpsimd.dma_start(h2T[:], c_zero[:, 0 : KT * B])
            nc.vector.memset(c1[:], 0.0)
            nc.vector.memset(c2[:], 0.0)

            ohlo = ohhi = None
            for t in range(t_steps):
                j = t % CH
                if j == 0:
                    nch = min(CH, t_steps - t)
                    ohlo = oh.tile([128, CH * B], DTR, tag="ohlo")
                    ohhi = oh.tile([128, CH * B], DTR, tag="ohhi")
                    nc.sync.dma_start(
                        ohlo[:, 0 : nch * B], p_oh[0, :, t * B : (t + nch) * B]
                    )
                    nc.sync.dma_start(
                        ohhi[:, 0 : nch * B], p_oh[1, :, t * B : (t + nch) * B]
                    )

                # ---- layer 1
                z1 = zp.tile([64, GC], DT, tag="z1")
                nc.tensor.matmul(
                    z1[:], r(ones_sb[:]), r(b0_sb[:]), start=True, stop=False
                )
                nc.tensor.matmul(
                    z1[:], r(ohlo[:, j * B : (j + 1) * B]), r(wie_sb[:, 0:GC]),
                    start=False, stop=False,
                )
                nc.tensor.matmul(
                    z1[:], r(ohhi[:, j * B : (j + 1) * B]), r(wie_sb[:, GC : 2 * GC]),
                    start=False, stop=False,
                )
                for k in range(KT):
                    nc.tensor.matmul(
                        z1[:],
                        r(h1T[:, k * B : (k + 1) * B]),
                        r(wh0_sb[:, k * GC : (k + 1) * GC]),
                        start=False,
                        stop=(k == KT - 1),
                    )
                h1o, c1 = cell(z1, c1, "1")
                h1T = gather(h1o)

                # ---- layer 2
                z2 = zp.tile([64, GC], DT, tag="z2")
                nc.tensor.matmul(
                    z2[:], r(ones_sb[:]), r(b1_sb[:]), start=True, stop=False
                )
                for k in range(KT):
                    nc.tensor.matmul(
                        z2[:],
                        r(h1T[:, k * B : (k + 1) * B]),
                        r(wi1_sb[:, k * GC : (k + 1) * GC]),
                        start=False,
                        stop=False,
                    )
                for k in range(KT):
                    nc.tensor.matmul(
                        z2[:],
                        r(h2T[:, k * B : (k + 1) * B]),
                        r(wh1_sb[:, k * GC : (k + 1) * GC]),
                        start=False,
                        stop=(k == KT - 1),
                    )
                h2o, c2 = cell(z2, c2, "2")
                h2T = gather(h2o)

                # ---- projection (redundant on every core)
                pj = pps2.tile([64, V], DT, tag="pj")
                for k in range(KT):
                    nc.tensor.matmul(
                        pj[:],
                        r(h2T[:, k * B : (k + 1) * B]),
                        r(wproj_sb[:, k * V : (k + 1) * V]),
                        start=(k == 0),
                        stop=(k == KT - 1),
                    )
                lo = tp.tile([64, V], DT, tag="lo")
                nc.vector.tensor_copy(lo[:], pj[:])
                nc.sync.dma_start(p_out[:, t, :], lo[:])

    for fk in fake_incs:
        si = fk.ins.sync_info
        neutered = False
        for u in si.on_update:
            if u.ant_name == "harr":
                u.update_value = 0
                neutered = True
        assert neutered, f"harr inc lost on {fk.ins}"
        fk.ins.sync_info = si
    nc.compile()
    return nc


def prep_inputs(idx, embed, Wi, Wh, b, Wproj, t_steps=T):
    """Host-side sharding/layout. Returns per-core in_maps."""
    order = [0, 1, 3, 2]  # i, f, o, g
    # sigma(x) = 0.5*(1 + tanh(x/2)): prescale i,f,o gate columns by 0.5 so a
    # single tanh activation covers all four gates on device.
    sc = np.concatenate([np.full(384, 0.5, np.float32), np.ones(128, np.float32)])

    def mov(a):  # (1024, 512) -> (128, 8*512) k-tile moving layout
        return np.ascontiguousarray(
            a.reshape(KT, 128, -1).transpose(1, 0, 2).reshape(128, -1)
        )

    idxf = idx[:, :t_steps].T.reshape(-1)  # (T*B,) t-major
    onehot = (
        (idxf[None, :] == np.arange(V, dtype=idxf.dtype)[:, None])
        .astype(np.float32)
        .reshape(2, 128, t_steps * B)
    )
    wproj = mov(Wproj)
    in_maps = []
    for c in range(NC):
        cols = np.concatenate(
            [np.arange(q * H + c * HC, q * H + (c + 1) * HC) for q in order]
        )
        m = {
            "embed": np.ascontiguousarray(embed),
            "wi0": mov(Wi[0][:, cols] * sc),
            "wh0": mov(Wh[0][:, cols] * sc),
            "wi1": mov(Wi[1][:, cols] * sc),
            "wh1": mov(Wh[1][:, cols] * sc),
            "b0": np.ascontiguousarray(b[0][cols] * sc).reshape(1, GC),
            "b1": np.ascontiguousarray(b[1][cols] * sc).reshape(1, GC),
            "wproj": wproj,
            "onehot": onehot,
        }
        in_maps.append({k: v.astype(v.dtype, copy=False) for k, v in m.items()})
    return in_maps


_NC_CACHE = {}


def _get_nc(t_steps, reps, comm="ag"):
    key = (t_steps, reps, comm)
    if key not in _NC_CACHE:
        _NC_CACHE[key] = build_nc(t_steps, reps, comm)
    return _NC_CACHE[key]


def run(idx, embed, Wi, Wh, b, Wproj, t_steps=T, reps=1, comm="wave"):
    nc = _get_nc(t_steps, reps, comm)
    in_maps = prep_inputs(idx, embed, Wi, Wh, b, Wproj, t_steps)
    res = run_bass_kernel_spmd(nc, in_maps, core_ids=list(range(NC)))
    return res.results[0]["out"]


def kernel(idx, embed, Wi, Wh, b, Wproj):
    out = run(
        np.asarray(idx), np.asarray(embed), np.asarray(Wi), np.asarray(Wh),
        np.asarray(b), np.asarray(Wproj),
    )
    return np.asarray(out, dtype=np.float32)

